# revision 1
# baseline (speedup 1.0000x reference)
"""Trainium2 Bass kernel for nn_KANSplineLayer (KAN spline layer, 8-core SPMD).

Math rewrite (validated to 6e-7 L2 rel err vs reference in fp32):
  reference: out = silu(BN_b(x @ Wb)) + BN_s(basis(minmax(x)) @ Ws.T)
  with 9 wide triangle-basis functions per input feature.

  Because each per-(o,i) spline g(z) = sum_k w[o,i,k]*tri_k(z) is continuous
  piecewise-linear on z in [0,1] with breakpoints {0,.25,.5,.75,1}, it equals
  a linear combination of {t, relu(t-1), relu(t-2), relu(t-3), 1} with
  t = 4*z in [0,4).  This shrinks the spline GEMM contraction from
  256*9=2304 to 256*4=1024 (+1 bias rank-1 term) and turns the basis
  construction into 1-op-per-plane elementwise work.

Sharding: data-parallel over rows (batch*H*W = 32768 -> 4096 rows/core).
Global per-feature min/max via a [128,4] AllReduce(min) on (min, -max).

Device pipeline per core:
  phase 1: DMA x tiles -> DVE stage -> PE transpose -> x^T in SBUF
           + DVE min/max reduction over rows
  collective: AllReduce(min) of [min | -max]
  phase 2: planes t = (x^T - min)*s4, r_m = relu(t - m)  (DVE/ACT)
           GEMMs (fp32r, full PE rate) into PSUM [rows, spline|base]
           epilogue: silu(base half) + spline half -> out rows
All PE matmul/transpose operands are produced by DVE so each PE
instruction needs at most one semaphore wait (walrus S3_LW limit).
"""
import numpy as np

import concourse.bacc as bacc
import concourse.bass as bass
import concourse.tile as tile
from concourse import mybir
from concourse.bass_utils import run_bass_kernel_spmd

# ---- problem constants (hardcoded; kernel.py must be self-contained) ----
IN_F, OUT_F = 256, 256
K_KNOTS = 9
EPS_MINMAX = 1e-7
EPS_BN = 1e-3
B, H, W = 32, 32, 32
N_TOTAL = B * H * W            # 32768 rows
N_CORES = 8
N_SHARD = N_TOTAL // N_CORES   # 4096 rows per core
R_TILES = N_SHARD // 128       # 32 row tiles per core
CH = 512                       # phase-2 column chunk (rows of output)
N_CHUNKS = N_SHARD // CH

F32 = mybir.dt.float32
MM_DT = mybir.dt.float32r      # full-rate fp32 matmul mode (N>=256)


def _host_prep(base_weight, spline_weight, spline_scaler,
               bn_base_gamma, bn_base_beta, bn_base_mean, bn_base_var,
               bn_spline_gamma, bn_spline_beta, bn_spline_mean, bn_spline_var):
    """Fold BN + rewrite spline into relu-plane weights. All in float64."""
    f64 = np.float64
    w = np.asarray(spline_weight, f64) * np.asarray(spline_scaler, f64)[:, :, None]
    knots = np.linspace(-1.0, 1.0, K_KNOTS).astype(f64)
    jg = np.arange(5, dtype=f64) / 4.0
    tri = np.maximum(0.0, 1.0 - np.abs(jg[None, :] - knots[:, None]))   # [k, j]
    G = np.einsum('oik,kj->oij', w, tri)                                # [o,i,5]
    a_s = np.asarray(bn_spline_gamma, f64) / np.sqrt(np.asarray(bn_spline_var, f64) + EPS_BN)
    b_s = np.asarray(bn_spline_beta, f64) - a_s * np.asarray(bn_spline_mean, f64)
    G = G * a_s[:, None, None]
    W_t = (G[:, :, 1] - G[:, :, 0]).T                                   # [i,o]
    H1 = (G[:, :, 2] - 2 * G[:, :, 1] + G[:, :, 0]).T
    H2 = (G[:, :, 3] - 2 * G[:, :, 2] + G[:, :, 1]).T
    H3 = (G[:, :, 4] - 2 * G[:, :, 3] + G[:, :, 2]).T
    C_s = G[:, :, 0].sum(axis=1) + b_s                                  # [o]
    a_b = np.asarray(bn_base_gamma, f64) / np.sqrt(np.asarray(bn_base_var, f64) + EPS_BN)
    b_b = np.asarray(bn_base_beta, f64) - a_b * np.asarray(bn_base_mean, f64)
    Wb = np.asarray(base_weight, f64) * a_b[None, :]                    # [i,o]
    f32 = np.float32
    w_t = np.stack([W_t[b * 128:(b + 1) * 128] for b in range(2)]).astype(f32)
    w_base = np.stack([Wb[b * 128:(b + 1) * 128] for b in range(2)]).astype(f32)
    w_r = np.stack([
        np.stack([Hm[b * 128:(b + 1) * 128] for b in range(2)])
        for Hm in (H1, H2, H3)]).astype(f32)                            # [3,2,128,256]
    bias_row = np.concatenate([C_s, b_b]).astype(f32)[None, :]          # [1,512]
    return w_t, w_base, w_r, bias_row


def _build_bass():
    nc = bacc.Bacc(num_devices=N_CORES)
    x_sh = nc.declare_dram_parameter("x_sh", [N_SHARD, IN_F], F32, isOutput=False)
    w_t_d = nc.declare_dram_parameter("w_t", [2, 128, 256], F32, isOutput=False)
    w_b_d = nc.declare_dram_parameter("w_base", [2, 128, 256], F32, isOutput=False)
    w_r_d = nc.declare_dram_parameter("w_r", [3, 2, 128, 256], F32, isOutput=False)
    bias_d = nc.declare_dram_parameter("bias_row", [1, 512], F32, isOutput=False)
    ident_d = nc.declare_dram_parameter("ident", [128, 128], F32, isOutput=False)
    out_sh = nc.declare_dram_parameter("out_sh", [N_SHARD, OUT_F], F32, isOutput=True)

    from contextlib import ExitStack
    with tile.TileContext(nc) as tc, ExitStack() as es:
        cons = es.enter_context(tc.tile_pool(name="cons", bufs=1))
        stage = es.enter_context(tc.tile_pool(name="stage", bufs=3))
        xin_p = es.enter_context(tc.tile_pool(name="xin", bufs=3))
        psT = es.enter_context(tc.tile_pool(name="psT", bufs=4, space="PSUM"))
        psM = es.enter_context(tc.tile_pool(name="psM", bufs=4, space="PSUM"))
        planes_p = es.enter_context(tc.tile_pool(name="planes", bufs=2))
        outp = es.enter_context(tc.tile_pool(name="outp", bufs=4))
        dram = es.enter_context(tc.tile_pool(name="dram", bufs=2, space="DRAM"))
        if True:
            # ---- constants, staged through DVE so PE waits stay single-sem ----
            def dve_load(nm, shape, dram_ap, dt=MM_DT):
                tmp = stage.tile(shape, F32, tag="ldtmp", name=f"ld_{nm}")
                nc.sync.dma_start(out=tmp[:], in_=dram_ap)
                t = cons.tile(shape, dt, tag=nm, name=nm)
                nc.vector.tensor_copy(out=t[:], in_=tmp[:])
                return t

            ident = dve_load("ident", [128, 128], ident_d[:], dt=F32)
            wt_sb = dve_load("wt_sb", [128, 2, 256], w_t_d.rearrange("b p n -> p b n"))
            wb_sb = dve_load("wb_sb", [128, 2, 256], w_b_d.rearrange("b p n -> p b n"))
            wr = dve_load("wr", [128, 3, 2, 256], w_r_d.rearrange("m b p n -> p m b n"))
            bias_sb = dve_load("bias_sb", [1, 512], bias_d[:])
            ones_f32 = cons.tile([1, 128], F32)
            nc.vector.memset(ones_f32[:], 1.0)
            ones = cons.tile([1, 128], MM_DT)
            nc.vector.tensor_copy(out=ones[:], in_=ones_f32[:])
            rb = cons.tile([128, 2], F32)     # ACT Relu biases -1, -2
            nc.vector.memset(rb[:, 0:1], -1.0)
            nc.vector.memset(rb[:, 1:2], -2.0)

            # x^T, feature blocks on partitions; fp32r so it can feed base GEMMs
            xt = cons.tile([128, 2, N_SHARD], MM_DT)

            # ---- phase 1: load + transpose + local min/max ----
            for r in range(R_TILES):
                xin = xin_p.tile([128, IN_F], F32)
                nc.sync.dma_start(out=xin[:], in_=x_sh[r * 128:(r + 1) * 128, :])
                xst = stage.tile([128, IN_F], F32, tag="xst")
                nc.vector.tensor_copy(out=xst[:], in_=xin[:])
                for b in range(2):
                    pst = psT.tile([128, 128], F32)
                    nc.tensor.transpose(pst[:], xst[:, b * 128:(b + 1) * 128], ident[:])
                    nc.vector.tensor_copy(
                        out=xt[:, b, r * 128:(r + 1) * 128], in_=pst[:])

            mm_loc = cons.tile([128, 4], F32)   # [min0, min1, -max0, -max1]
            lmax = cons.tile([128, 2], F32)
            for b in range(2):
                nc.vector.tensor_reduce(
                    out=mm_loc[:, b:b + 1], in_=xt[:, b, :],
                    op=mybir.AluOpType.min, axis=mybir.AxisListType.X)
                nc.vector.tensor_reduce(
                    out=lmax[:, b:b + 1], in_=xt[:, b, :],
                    op=mybir.AluOpType.max, axis=mybir.AxisListType.X)
            nc.vector.tensor_scalar(
                out=mm_loc[:, 2:4], in0=lmax[:], scalar1=-1.0, scalar2=None,
                op0=mybir.AluOpType.mult)

            # ---- global min/max across the 8 cores ----
            cc_in = dram.tile([128, 4], F32)
            cc_out = dram.tile([128, 4], F32)
            nc.sync.dma_start(out=cc_in[:], in_=mm_loc[:])
            nc.gpsimd.collective_compute(
                "AllReduce", mybir.AluOpType.min,
                replica_groups=[list(range(N_CORES))],
                ins=[cc_in.opt()], outs=[cc_out.opt()])
            gmm = cons.tile([128, 4], F32)       # [gmin0, gmin1, -gmax0, -gmax1]
            nc.sync.dma_start(out=gmm[:], in_=cc_out[:])

            # s4 = 4/(gmax-gmin+eps); t = (x - gmin)*s4
            nrng = cons.tile([128, 2], F32)
            qt = cons.tile([128, 2], F32)
            s4 = cons.tile([128, 2], F32)
            for b in range(2):
                nc.vector.tensor_tensor(
                    out=nrng[:, b:b + 1], in0=gmm[:, b:b + 1],
                    in1=gmm[:, 2 + b:3 + b], op=mybir.AluOpType.add)  # gmin-gmax
            nc.vector.tensor_scalar(
                out=qt[:], in0=nrng[:], scalar1=-0.25, scalar2=EPS_MINMAX * 0.25,
                op0=mybir.AluOpType.mult, op1=mybir.AluOpType.add)
            nc.vector.reciprocal(out=s4[:], in_=qt[:])

            # ---- phase 2: planes + GEMMs + epilogue ----
            for c in range(N_CHUNKS):
                cs = slice(c * CH, (c + 1) * CH)
                tpl = [planes_p.tile([128, CH], MM_DT, tag=f"t{b}", name=f"t{b}_{c}")
                       for b in range(2)]
                rpl = [[planes_p.tile([128, CH], MM_DT, tag=f"r{m}{b}", name=f"r{m}{b}_{c}")
                        for b in range(2)] for m in range(3)]
                for b in range(2):
                    # t = (x^T - gmin) * s4   (DVE, per-partition scalars)
                    nc.vector.tensor_scalar(
                        out=tpl[b][:], in0=xt[:, b, cs],
                        scalar1=gmm[:, b:b + 1], scalar2=s4[:, b:b + 1],
                        op0=mybir.AluOpType.subtract, op1=mybir.AluOpType.mult)
                    # r1/r2 on ACT, r3 on DVE
                    for m in (1, 2):
                        nc.scalar.activation(
                            out=rpl[m - 1][b][:], in_=tpl[b][:],
                            func=mybir.ActivationFunctionType.Relu,
                            bias=rb[:, m - 1:m], scale=1.0)
                    nc.vector.tensor_scalar(
                        out=rpl[2][b][:], in0=tpl[b][:], scalar1=3.0, scalar2=0.0,
                        op0=mybir.AluOpType.subtract, op1=mybir.AluOpType.max)
                for j in range(CH // 128):
                    js = slice(j * 128, (j + 1) * 128)
                    ps = psM.tile([128, 512], F32)
                    # rank-1 bias: ones^T @ [C_s | b_b]
                    nc.tensor.matmul(
                        ps[:], ones[:], bias_sb[:],
                        start=True, stop=False, skip_group_check=True)
                    for b in range(2):
                        nc.tensor.matmul(
                            ps[:, 0:256], tpl[b][:, js], wt_sb[:, b, :],
                            start=False, stop=False, skip_group_check=True)
                        nc.tensor.matmul(
                            ps[:, 256:512], xt[:, b, c * CH + j * 128:c * CH + (j + 1) * 128],
                            wb_sb[:, b, :],
                            start=False, stop=False, skip_group_check=True)
                    for m in range(3):
                        for b in range(2):
                            nc.tensor.matmul(
                                ps[:, 0:256], rpl[m][b][:, js], wr[:, m, b, :],
                                start=False, stop=(m == 2 and b == 1),
                                skip_group_check=True)
                    o = outp.tile([128, OUT_F], F32)
                    nc.scalar.activation(
                        out=o[:], in_=ps[:, 256:512],
                        func=mybir.ActivationFunctionType.Silu)
                    nc.vector.tensor_tensor(
                        out=o[:], in0=o[:], in1=ps[:, 0:256],
                        op=mybir.AluOpType.add)
                    r0 = c * CH + j * 128
                    nc.sync.dma_start(out=out_sh[r0:r0 + 128, :], in_=o[:])
    nc.compile()
    return nc


_CACHE = {}


def make_in_maps(inputs):
    x = np.ascontiguousarray(np.asarray(inputs["x"], np.float32))
    w_t, w_base, w_r, bias_row = _host_prep(
        **{k: v for k, v in inputs.items() if k != "x"})
    ident = np.eye(128, dtype=np.float32)
    xf = x.reshape(N_TOTAL, IN_F)
    return [{
        "x_sh": np.ascontiguousarray(xf[c * N_SHARD:(c + 1) * N_SHARD]),
        "w_t": w_t, "w_base": w_base, "w_r": w_r, "bias_row": bias_row,
        "ident": ident,
    } for c in range(N_CORES)]


def kernel(**inputs):
    if "nc" not in _CACHE:
        _CACHE["nc"] = _build_bass()
    nc = _CACHE["nc"]
    in_maps = make_in_maps(inputs)
    res = run_bass_kernel_spmd(nc, in_maps, list(range(N_CORES)))
    out = np.concatenate([res.results[c]["out_sh"] for c in range(N_CORES)], axis=0)
    return out.reshape(B, H, W, OUT_F).astype(np.float32)



# revision 7
# speedup vs baseline: 1.2399x; 1.2399x over previous
"""Trainium2 Bass kernel for nn_KANSplineLayer (KAN spline layer, 8-core SPMD).

Math rewrite (validated 3.7e-3 rel err vs reference with bf16 operands):
  reference: out = silu(BN_b(x @ Wb)) + BN_s(basis(minmax(x)) @ Ws.T)
  Spline is CPWL on t = 4*xn in [0,4] with kinks at {1,2,3}:
    spline = t@Wt + relu(t-1)@H1 + relu(t-2)@H2 + relu(t-3)@H3 + C_s
  (contraction 4*256 instead of 9*256). All GEMM operands bf16 (full PE
  rate; fp32r streams at half rate), accumulate fp32 in PSUM.

Schedule (the point of this version): the 2KB AllReduce(min) has a ~34us
trigger-to-done latency floor, so trigger it as early as possible and fill
its window:
  phase 1a (0..~8us): big-block DMA of x (bf16), running pairwise min/max
      on the UNtransposed tiles (DVE TT) -> 4 small PE transposes of the
      [128,256] accumulators -> free-axis reduce -> mm_loc -> AllReduce.
  AR window (~8..~42us): 64 PE transposes of x -> x^T (bf16), PSUM
      evacuations, full base path (GEMM + b_b rank-1 + SiLU + C_s add)
      producing sbase' = silu(x@Wb + b_b) + C_s per row tile.
  phase 2 (post-AR): planes t,r1,r2,r3 straight from x^T on ACT
      (per-partition scale=s4, bias=-gmin*s4-m), 8 bf16 matmuls per
      128-row tile into PSUM, epilogue add sbase', DMA out (f32).
"""
import numpy as np
import ml_dtypes

import concourse.bacc as bacc
import concourse.bass as bass
import concourse.tile as tile
from concourse import mybir
from concourse.bass_utils import run_bass_kernel_spmd

# ---- problem constants (hardcoded; kernel.py must be self-contained) ----
IN_F, OUT_F = 256, 256
K_KNOTS = 9
EPS_MINMAX = 1e-7
EPS_BN = 1e-3
B, H, W = 32, 32, 32
N_TOTAL = B * H * W            # 32768 rows
N_CORES = 8
N_SHARD = N_TOTAL // N_CORES   # 4096 rows per core
R_TILES = N_SHARD // 128       # 32 row tiles per core
G_TILES = 4                    # row tiles per input DMA group
N_GROUPS = R_TILES // G_TILES  # 8 input DMA groups
CH = 512                       # phase-2 column chunk (rows of output)
N_CHUNKS = N_SHARD // CH

F32 = mybir.dt.float32
BF16 = mybir.dt.bfloat16
NPBF16 = ml_dtypes.bfloat16


def _host_prep(base_weight, spline_weight, spline_scaler,
               bn_base_gamma, bn_base_beta, bn_base_mean, bn_base_var,
               bn_spline_gamma, bn_spline_beta, bn_spline_mean, bn_spline_var):
    """Fold BN + rewrite spline into relu-plane weights. All in float64."""
    f64 = np.float64
    w = np.asarray(spline_weight, f64) * np.asarray(spline_scaler, f64)[:, :, None]
    knots = np.linspace(-1.0, 1.0, K_KNOTS).astype(f64)
    jg = np.arange(5, dtype=f64) / 4.0
    tri = np.maximum(0.0, 1.0 - np.abs(jg[None, :] - knots[:, None]))   # [k, j]
    G = np.einsum('oik,kj->oij', w, tri)                                # [o,i,5]
    a_s = np.asarray(bn_spline_gamma, f64) / np.sqrt(np.asarray(bn_spline_var, f64) + EPS_BN)
    b_s = np.asarray(bn_spline_beta, f64) - a_s * np.asarray(bn_spline_mean, f64)
    G = G * a_s[:, None, None]
    W_t = (G[:, :, 1] - G[:, :, 0]).T                                   # [i,o]
    H1 = (G[:, :, 2] - 2 * G[:, :, 1] + G[:, :, 0]).T
    H2 = (G[:, :, 3] - 2 * G[:, :, 2] + G[:, :, 1]).T
    H3 = (G[:, :, 4] - 2 * G[:, :, 3] + G[:, :, 2]).T
    C_s = G[:, :, 0].sum(axis=1) + b_s                                  # [o]
    a_b = np.asarray(bn_base_gamma, f64) / np.sqrt(np.asarray(bn_base_var, f64) + EPS_BN)
    b_b = np.asarray(bn_base_beta, f64) - a_b * np.asarray(bn_base_mean, f64)
    Wb = np.asarray(base_weight, f64) * a_b[None, :]                    # [i,o]

    def blk(M):  # [256, 256] -> [128, 2, 256]
        return np.stack([M[0:128], M[128:256]], axis=1)

    bf = NPBF16
    wt_host = blk(W_t).astype(bf)                                       # [128,2,256]
    wb_host = blk(Wb).astype(bf)
    wr_host = np.stack([blk(Hm) for Hm in (H1, H2, H3)], axis=1).astype(bf)  # [128,3,2,256]
    bb_row = np.asarray(b_b, f64)[None, :].astype(bf)                   # [1,256]
    csb_host = np.tile(np.asarray(C_s, f64)[None, :], (128, 1)).astype(bf)  # [128,256]
    return wt_host, wb_host, wr_host, bb_row, csb_host


def _build_bass():
    nc = bacc.Bacc(num_devices=N_CORES)
    x_sh = nc.declare_dram_parameter("x_sh", [N_SHARD, IN_F], BF16, isOutput=False)
    w_t_d = nc.declare_dram_parameter("w_t", [128, 2, 256], BF16, isOutput=False)
    w_b_d = nc.declare_dram_parameter("w_base", [128, 2, 256], BF16, isOutput=False)
    w_r_d = nc.declare_dram_parameter("w_r", [128, 3, 2, 256], BF16, isOutput=False)
    bb_d = nc.declare_dram_parameter("bb_row", [1, 256], BF16, isOutput=False)
    csb_d = nc.declare_dram_parameter("csb", [128, 256], BF16, isOutput=False)
    ident_d = nc.declare_dram_parameter("ident", [128, 128], BF16, isOutput=False)
    out_sh = nc.declare_dram_parameter("out_sh", [N_SHARD, OUT_F], F32, isOutput=True)

    x_g = x_sh.rearrange("(g t p) f -> g p t f", g=N_GROUPS, t=G_TILES, p=128)

    from contextlib import ExitStack
    with tile.TileContext(nc) as tc, ExitStack() as es:
        cons = es.enter_context(tc.tile_pool(name="cons", bufs=1))
        stage = es.enter_context(tc.tile_pool(name="stage", bufs=2))
        xin_p = es.enter_context(tc.tile_pool(name="xin", bufs=N_GROUPS))
        psT = es.enter_context(tc.tile_pool(name="psT", bufs=4, space="PSUM"))
        psB = es.enter_context(tc.tile_pool(name="psB", bufs=2, space="PSUM"))
        psM = es.enter_context(tc.tile_pool(name="psM", bufs=2, space="PSUM"))
        planes_p = es.enter_context(tc.tile_pool(name="planes", bufs=2))
        outp = es.enter_context(tc.tile_pool(name="outp", bufs=4))
        dram = es.enter_context(tc.tile_pool(name="dram", bufs=2, space="DRAM"))

        # ---- tiny consts first: identity for PE transposes ----
        id_st = stage.tile([128, 128], BF16, tag="id_st")
        nc.sync.dma_start(out=id_st[:], in_=ident_d[:])
        ident = cons.tile([128, 128], BF16, tag="ident")
        nc.scalar.copy(out=ident[:], in_=id_st[:])
        ones = cons.tile([1, 128], BF16, tag="ones")
        nc.vector.memset(ones[:], 1.0)

        # ---- phase 1a: x DMA (big groups) + running min/max on DVE ----
        xins = []
        accmin = cons.tile([128, G_TILES * IN_F], BF16, tag="accmin")
        accmax = cons.tile([128, G_TILES * IN_F], BF16, tag="accmax")
        for g in range(N_GROUPS):
            xin = xin_p.tile([128, G_TILES, IN_F], BF16, tag="xin", name=f"xin{g}")
            nc.sync.dma_start(out=xin[:], in_=x_g[g])
            xins.append(xin)
            v = xin[:].rearrange("p t f -> p (t f)")
            if g == 0:
                nc.vector.tensor_copy(out=accmin[:], in_=v)
                nc.vector.tensor_copy(out=accmax[:], in_=v)
            else:
                nc.vector.tensor_tensor(out=accmin[:], in0=accmin[:], in1=v,
                                        op=mybir.AluOpType.min)
                nc.vector.tensor_tensor(out=accmax[:], in0=accmax[:], in1=v,
                                        op=mybir.AluOpType.max)
        # tree-combine [128,1024] -> [128,256]
        m512 = cons.tile([128, 2, 512], BF16, tag="m512")
        m256 = cons.tile([128, 2, 256], BF16, tag="m256")  # [:,0]=min, [:,1]=max
        for i, acc in enumerate((accmin, accmax)):
            op = mybir.AluOpType.min if i == 0 else mybir.AluOpType.max
            nc.vector.tensor_tensor(out=m512[:, i], in0=acc[:, 0:512],
                                    in1=acc[:, 512:1024], op=op)
            nc.vector.tensor_tensor(out=m256[:, i], in0=m512[:, i, 0:256],
                                    in1=m512[:, i, 256:512], op=op)
        # transpose accumulators, reduce over the 128 partial rows
        mm_loc = cons.tile([128, 4], F32, tag="mm_loc")  # [min0,min1,-max0,-max1]
        lmax = cons.tile([128, 2], F32, tag="lmax")
        for i in range(2):           # 0=min, 1=max
            op = mybir.AluOpType.min if i == 0 else mybir.AluOpType.max
            for b in range(2):       # feature block
                pc = psT.tile([128, 128], BF16, tag="pst")
                nc.tensor.transpose(pc[:], m256[:, i, b * 128:(b + 1) * 128], ident[:])
                dst = mm_loc[:, b:b + 1] if i == 0 else lmax[:, b:b + 1]
                nc.vector.tensor_reduce(out=dst, in_=pc[:], op=op,
                                        axis=mybir.AxisListType.X)
        nc.vector.tensor_scalar(
            out=mm_loc[:, 2:4], in0=lmax[:], scalar1=-1.0, scalar2=None,
            op0=mybir.AluOpType.mult)

        # ---- weights (DMA after x; staged via ACT so DVE stays free) ----
        def act_load(nm, shape, dram_ap):
            tmp = stage.tile(shape, BF16, tag=f"st_{nm}", name=f"st_{nm}")
            nc.sync.dma_start(out=tmp[:], in_=dram_ap)
            t = cons.tile(shape, BF16, tag=nm, name=nm)
            nc.scalar.copy(out=t[:], in_=tmp[:])
            return t

        wt_sb = act_load("wt_sb", [128, 2, 256], w_t_d[:])
        wb_sb = act_load("wb_sb", [128, 2, 256], w_b_d[:])
        wr = act_load("wr", [128, 3, 2, 256], w_r_d[:])
        bb_row = act_load("bb_row", [1, 256], bb_d[:])
        csb = act_load("csb", [128, 256], csb_d[:])

        # ---- global min/max across the 8 cores ----
        cc_in = dram.tile([128, 4], F32)
        cc_out = dram.tile([128, 4], F32)
        nc.sync.dma_start(out=cc_in[:], in_=mm_loc[:])
        nc.gpsimd.collective_compute(
            "AllReduce", mybir.AluOpType.min,
            replica_groups=[list(range(N_CORES))],
            ins=[cc_in.opt()], outs=[cc_out.opt()])
        gmm = cons.tile([128, 4], F32, tag="gmm")  # [gmin0,gmin1,-gmax0,-gmax1]
        nc.sync.dma_start(out=gmm[:], in_=cc_out[:])

        # ---- AR window: transposes -> x^T, then full base path ----
        xt = cons.tile([128, 2, N_SHARD], BF16, tag="xt")
        for r in range(R_TILES):
            g, t = divmod(r, G_TILES)
            for b in range(2):
                pst = psT.tile([128, 128], BF16, tag="pst")
                nc.tensor.transpose(
                    pst[:], xins[g][:, t, b * 128:(b + 1) * 128], ident[:])
                nc.vector.tensor_copy(
                    out=xt[:, b, r * 128:(r + 1) * 128], in_=pst[:])

        sb = cons.tile([128, R_TILES, OUT_F], BF16, tag="sb")  # silu(base)+C_s
        for r in range(R_TILES):
            rs = slice(r * 128, (r + 1) * 128)
            pb = psB.tile([128, 256], F32, tag="psb")
            nc.tensor.matmul(pb[:], xt[:, 0, rs], wb_sb[:, 0, :],
                             start=True, stop=False, skip_group_check=True)
            nc.tensor.matmul(pb[:], xt[:, 1, rs], wb_sb[:, 1, :],
                             start=False, stop=False, skip_group_check=True)
            nc.tensor.matmul(pb[:], ones[:], bb_row[:],
                             start=False, stop=True, skip_group_check=True)
            nc.scalar.activation(out=sb[:, r, :], in_=pb[:],
                                 func=mybir.ActivationFunctionType.Silu)
            nc.vector.tensor_tensor(out=sb[:, r, :], in0=sb[:, r, :], in1=csb[:],
                                    op=mybir.AluOpType.add)

        # ---- post-AR scalars: s4 = 4/(range+eps), plane biases ----
        nrng = cons.tile([128, 2], F32, tag="nrng")
        qt = cons.tile([128, 2], F32, tag="qt")
        s4 = cons.tile([128, 2], F32, tag="s4")
        gs = cons.tile([128, 2], F32, tag="gs")
        cb = cons.tile([128, 2, 4], F32, tag="cb")  # [:,b,m]: -gs-m (m=0: t bias)
        for b in range(2):
            nc.vector.tensor_tensor(
                out=nrng[:, b:b + 1], in0=gmm[:, b:b + 1],
                in1=gmm[:, 2 + b:3 + b], op=mybir.AluOpType.add)  # gmin-gmax
        nc.vector.tensor_scalar(
            out=qt[:], in0=nrng[:], scalar1=-0.25, scalar2=EPS_MINMAX * 0.25,
            op0=mybir.AluOpType.mult, op1=mybir.AluOpType.add)
        nc.vector.reciprocal(out=s4[:], in_=qt[:])
        nc.vector.tensor_tensor(out=gs[:], in0=gmm[:, 0:2], in1=s4[:],
                                op=mybir.AluOpType.mult)
        for m in range(4):
            nc.vector.tensor_scalar(
                out=cb[:, :, m], in0=gs[:], scalar1=-1.0, scalar2=-float(m),
                op0=mybir.AluOpType.mult, op1=mybir.AluOpType.add)

        # ---- phase 2: planes (ACT) + spline GEMMs + epilogue ----
        AF = mybir.ActivationFunctionType
        for c in range(N_CHUNKS):
            cs = slice(c * CH, (c + 1) * CH)
            tpl = [[None, None] for _ in range(4)]  # [plane m][block b]
            for m in range(4):
                func = AF.Identity if m == 0 else AF.Relu
                for b in range(2):
                    p = planes_p.tile([128, CH], BF16, tag=f"p{m}{b}",
                                      name=f"p{m}{b}_{c}")
                    nc.scalar.activation(
                        out=p[:], in_=xt[:, b, cs], func=func,
                        bias=cb[:, b, m:m + 1], scale=s4[:, b:b + 1])
                    tpl[m][b] = p
            for j in range(CH // 128):
                js = slice(j * 128, (j + 1) * 128)
                ps = psM.tile([128, 256], F32, tag="psm")
                first = True
                for m in range(4):
                    for b in range(2):
                        rhs = wt_sb[:, b, :] if m == 0 else wr[:, m - 1, b, :]
                        nc.tensor.matmul(
                            ps[:], tpl[m][b][:, js], rhs,
                            start=first, stop=(m == 3 and b == 1),
                            skip_group_check=True)
                        first = False
                r = c * (CH // 128) + j
                o = outp.tile([128, OUT_F], F32, tag="o", name=f"o_{r}")
                nc.vector.tensor_tensor(
                    out=o[:], in0=ps[:], in1=sb[:, r, :],
                    op=mybir.AluOpType.add)
                nc.sync.dma_start(out=out_sh[r * 128:(r + 1) * 128, :], in_=o[:])
    nc.compile()
    return nc


_CACHE = {}


def make_in_maps(inputs):
    w_t, w_base, w_r, bb_row, csb = _host_prep(
        **{k: v for k, v in inputs.items() if k != "x"})
    ident = np.eye(128, dtype=NPBF16)
    xf = np.asarray(inputs["x"], np.float32).reshape(N_TOTAL, IN_F).astype(NPBF16)
    return [{
        "x_sh": np.ascontiguousarray(xf[c * N_SHARD:(c + 1) * N_SHARD]),
        "w_t": w_t, "w_base": w_base, "w_r": w_r, "bb_row": bb_row,
        "csb": csb, "ident": ident,
    } for c in range(N_CORES)]


def kernel(**inputs):
    if "nc" not in _CACHE:
        _CACHE["nc"] = _build_bass()
    nc = _CACHE["nc"]
    in_maps = make_in_maps(inputs)
    res = run_bass_kernel_spmd(nc, in_maps, list(range(N_CORES)))
    out = np.concatenate([res.results[c]["out_sh"] for c in range(N_CORES)], axis=0)
    return out.reshape(B, H, W, OUT_F).astype(np.float32)
